# revision 18
# baseline (speedup 1.0000x reference)
"""HONU order-3 kernel for 8 TRN2 NeuronCores.

Math: out[b] = sum_{i<=j<=k} w_ijk * xf_i * xf_j * xf_k,  xf = [1, x] (127 feats).

Restructuring: group combos by pair (i,j) (lex order => per-pair weights are a
contiguous slice of `weights`).  Let W[(i,j), k] = w_ijk for k>=j (0 otherwise).
Then  Z[b,(i,j)] = sum_k W[(i,j),k] * xf[b,k]   (a dense matmul), and
      out[b]     = sum_{(i,j)} Q[b,(i,j)] * Z[b,(i,j)],   Q[b,(i,j)] = xf_i*xf_j.

Sharding: pair-rows i are dealt round-robin to the 8 cores (core c gets rows
i = 8t + c, t = 0..15); class t occupies columns [OFFS[t], OFFS[t+1]) covering
j in [8t, 128) (8-aligned; leading j in [8t,i) and j=127 carry zero weights).
NCOLS = 1088 per core.

The pair-products Q are BUILT ON THE HOST (they are pure input data) and
shipped as bf16, so the only on-chip epilogue work is two fused
multiply+accumulates (scalar_tensor_tensor) per 128-batch tile, reading Z
straight from PSUM.  Weights and Q are split at column 512 so the first dot
overlaps the remaining transfers (the 3 HWDGE queues serialize at the
~300GB/s per-core HBM cap, so transfer ORDER is what matters).  Matmuls run
in bf16 (total rel err ~1.7e-3, tolerance 2e-2).  res [128,2] is
PE-transposed to [2,128] so the output DMA is 2 contiguous 512B descriptors.

x is replicated; each core returns a [2,128] partial that the host sums.
"""

import numpy as np
import ml_dtypes

import concourse.bass as bass
import concourse.bacc as bacc
import concourse.tile as tile
import concourse.mybir as mybir
from concourse.bass_utils import run_bass_kernel_spmd
from concourse.masks import make_identity

F32 = mybir.dt.float32
BF16 = mybir.dt.bfloat16
BF16_NP = ml_dtypes.bfloat16

P = 128
NF = 127            # features incl. bias
B = 256             # batch
NCLASS = 16
WIDTHS = [128 - 8 * t for t in range(NCLASS)]
OFFS = np.concatenate([[0], np.cumsum(WIDTHS)])
NCOLS = int(OFFS[-1])                                   # 1088
SPLIT = 512                                             # dot-a | dot-b boundary
NB = NCOLS - SPLIT                                      # 576

_CACHE = {}


def _build_nc():
    nc = bacc.Bacc("TRN2", target_bir_lowering=False, debug=False)
    xt = nc.dram_tensor("xt", [P, B], BF16, kind="ExternalInput")   # xf^T padded
    wda = nc.dram_tensor("wda", [P, SPLIT], BF16, kind="ExternalInput")
    wdb = nc.dram_tensor("wdb", [P, NB], BF16, kind="ExternalInput")
    qh = [[nc.dram_tensor(f"qh{bt}{h}", [P, SPLIT if h == "a" else NB], BF16,
                          kind="ExternalInput") for h in "ab"]
          for bt in range(2)]
    out = nc.dram_tensor("out", [2, P], F32, kind="ExternalOutput")

    with tile.TileContext(nc) as tc:
        with (
            tc.tile_pool(name="const", bufs=1) as cpool,
            tc.tile_pool(name="ps", bufs=2, space="PSUM") as ps,
            tc.tile_pool(name="pst", bufs=1, space="PSUM") as pst,
        ):
            wda_t = cpool.tile([P, SPLIT], BF16, tag="wda")
            wdb_t = cpool.tile([P, NB], BF16, tag="wdb")
            xt_t = cpool.tile([P, B], BF16, tag="xt")
            qh_t = [[cpool.tile([P, SPLIT if h == "a" else NB], BF16,
                                tag=f"qh{bt}{h}", name=f"qh{bt}{h}_t")
                     for h in "ab"] for bt in range(2)]
            # transfers serialize across the 3 queues at the per-core HBM
            # cap, so issue in need-order: xt+wda gate the first matmul,
            # qh0a the first dot; qh1b is needed last
            nc.sync.dma_start(wda_t[:], wda[:])
            nc.sync.dma_start(wdb_t[:], wdb[:])
            nc.scalar.dma_start(xt_t[:], xt[:])
            nc.scalar.dma_start(qh_t[0][1][:], qh[0][1][:])
            nc.scalar.dma_start(qh_t[1][1][:], qh[1][1][:])
            nc.gpsimd.dma_start(qh_t[0][0][:], qh[0][0][:])
            nc.gpsimd.dma_start(qh_t[1][0][:], qh[1][0][:])

            ident = cpool.tile([P, P], F32, tag="ident")
            make_identity(nc, ident[:])

            res = cpool.tile([P, 2], F32, tag="res")
            acc = cpool.tile([P, 4], F32, tag="acc")
            e = cpool.tile([P, NB], F32, tag="e")
            for bt in range(2):
                z_ps = ps.tile([P, NCOLS], F32, tag="z", name=f"z{bt}_ps")
                xts = xt_t[:, bt * P:(bt + 1) * P]
                nc.tensor.matmul(z_ps[:, 0:512], xts, wda_t[:],
                                 start=True, stop=True)
                nc.vector.scalar_tensor_tensor(
                    out=e[:, 0:SPLIT], in0=z_ps[:, 0:SPLIT], scalar=1.0,
                    in1=qh_t[bt][0][:],
                    op0=mybir.AluOpType.mult, op1=mybir.AluOpType.mult,
                    accum_out=acc[:, 2 * bt:2 * bt + 1],
                )
                nc.tensor.matmul(z_ps[:, 512:1024], xts, wdb_t[:, 0:512],
                                 start=True, stop=True)
                nc.tensor.matmul(z_ps[:, 1024:NCOLS], xts, wdb_t[:, 512:NB],
                                 start=True, stop=True)
                nc.vector.scalar_tensor_tensor(
                    out=e[:], in0=z_ps[:, SPLIT:NCOLS], scalar=1.0,
                    in1=qh_t[bt][1][:],
                    op0=mybir.AluOpType.mult, op1=mybir.AluOpType.mult,
                    accum_out=acc[:, 2 * bt + 1:2 * bt + 2],
                )
            nc.vector.tensor_reduce(
                res[:], acc[:].rearrange("p (t h) -> p t h", t=2),
                axis=mybir.AxisListType.X, op=mybir.AluOpType.add,
            )
            # [128,2] -> [2,128] so the out DMA is 2 contiguous descriptors
            tps = pst.tile([2, P], F32, tag="tps")
            nc.tensor.transpose(tps[:], res[:], ident[:])
            osb = cpool.tile([2, P], F32, tag="osb")
            nc.vector.tensor_copy(osb[:], tps[:])
            # split over two queues: desc-gen runs in parallel
            nc.scalar.dma_start(out[0:1, :], osb[0:1, :])
            nc.sync.dma_start(out[1:2, :], osb[1:2, :])
    nc.compile()
    return nc


def _prep_inputs(x, weights, comb_idx):
    """Host-side layout prep: xf paddings, pair-products Q, dense weight chunks."""
    x = np.ascontiguousarray(np.asarray(x, dtype=np.float32))
    w = np.asarray(weights, dtype=np.float32).ravel()
    ci = np.asarray(comb_idx)
    i_, j_ = ci[:, 0].astype(np.int64), ci[:, 1].astype(np.int64)
    k_ = ci[:, 2].astype(np.int64)

    xf = np.concatenate([np.ones((B, 1), np.float32), x], axis=1)   # [256,127]
    xbp = np.zeros((B, P), np.float32)
    xbp[:, :NF] = xf

    xt = np.zeros((P, B), np.float32)
    xt[:NF, :] = xf.T
    xt16 = xt.astype(BF16_NP)

    # lex pair-row index of each combo
    ar = np.arange(NF, dtype=np.int64)
    rsp = ar * NF - (ar * (ar - 1)) // 2
    q = rsp[i_] + (j_ - i_)
    Wd = np.zeros((8128, NF), np.float32)
    Wd[q, k_] = w

    in_maps = []
    for c in range(8):
        big = np.zeros((P, NCOLS), np.float32)
        Q = np.zeros((B, NCOLS), np.float32)
        for t in range(NCLASS):
            i = 8 * t + c
            if i > 126:
                continue
            o = int(OFFS[t])
            Q[:, o:o + WIDTHS[t]] = xf[:, i:i + 1] * xbp[:, 8 * t:P]
            p0 = int(rsp[i])
            big[:NF, o + (i - 8 * t): o + (NF - 8 * t)] = Wd[p0:p0 + (NF - i)].T
        big16 = big.astype(BF16_NP)
        Q16 = Q.astype(BF16_NP)
        m = {
            "xt": xt16,
            "wda": np.ascontiguousarray(big16[:, 0:SPLIT]),
            "wdb": np.ascontiguousarray(big16[:, SPLIT:NCOLS]),
        }
        for bt in range(2):
            m[f"qh{bt}a"] = np.ascontiguousarray(Q16[bt * P:(bt + 1) * P, 0:SPLIT])
            m[f"qh{bt}b"] = np.ascontiguousarray(Q16[bt * P:(bt + 1) * P, SPLIT:NCOLS])
        in_maps.append(m)
    return in_maps


def _get_nc():
    if "nc" not in _CACHE:
        _CACHE["nc"] = _build_nc()
    return _CACHE["nc"]


def run_spmd(x, weights, comb_idx, trace=False):
    nc = _get_nc()
    in_maps = _prep_inputs(x, weights, comb_idx)
    res = run_bass_kernel_spmd(nc, in_maps, list(range(8)), trace=trace)
    acc = np.zeros((2, P), np.float64)
    for c in range(8):
        acc += res.results[c]["out"].astype(np.float64)
    return acc.reshape(B, 1).astype(np.float32), res


def kernel(x, weights, comb_idx):
    out, _ = run_spmd(x, weights, comb_idx, trace=False)
    return out


# revision 19
# speedup vs baseline: 1.0186x; 1.0186x over previous
"""HONU order-3 kernel for 8 TRN2 NeuronCores.

Math: out[b] = sum_{i<=j<=k} w_ijk * xf_i * xf_j * xf_k,  xf = [1, x] (127 feats).

Restructuring: group combos by pair (i,j) (lex order => per-pair weights are a
contiguous slice of `weights`).  Let W[(i,j), k] = w_ijk for k>=j (0 otherwise).
Then  Z[b,(i,j)] = sum_k W[(i,j),k] * xf[b,k]   (a dense matmul), and
      out[b]     = sum_{(i,j)} Q[b,(i,j)] * Z[b,(i,j)],   Q[b,(i,j)] = xf_i*xf_j.

Sharding: pair-rows i are dealt round-robin to the 8 cores (core c gets rows
i = 8t + c, t = 0..15); class t occupies columns [OFFS[t], OFFS[t+1]) covering
j in [8t, 128) (8-aligned; leading j in [8t,i) and j=127 carry zero weights).
NCOLS = 1088 per core.

The pair-products Q are BUILT ON THE HOST (they are pure input data) and
shipped as bf16, so the only on-chip epilogue work is two fused
multiply+accumulates (scalar_tensor_tensor) per 128-batch tile, reading Z
straight from PSUM.  Weights and Q are split at column 512 so the first dot
overlaps the remaining transfers (the 3 HWDGE queues serialize at the
~300GB/s per-core HBM cap, so transfer ORDER is what matters).  Matmuls run
in bf16 (total rel err ~1.7e-3, tolerance 2e-2).  res [128,2] is
PE-transposed to [2,128] so the output DMA is 2 contiguous 512B descriptors.

x is replicated; each core returns a [2,128] partial that the host sums.
"""

import numpy as np
import ml_dtypes

import concourse.bass as bass
import concourse.bacc as bacc
import concourse.tile as tile
import concourse.mybir as mybir
from concourse.bass_utils import run_bass_kernel_spmd
from concourse.masks import make_identity

F32 = mybir.dt.float32
BF16 = mybir.dt.bfloat16
BF16_NP = ml_dtypes.bfloat16

P = 128
NF = 127            # features incl. bias
B = 256             # batch
NCLASS = 16
WIDTHS = [128 - 8 * t for t in range(NCLASS)]
OFFS = np.concatenate([[0], np.cumsum(WIDTHS)])
NCOLS = int(OFFS[-1])                                   # 1088
SPLIT = 512                                             # dot-a | dot-b boundary
NB = NCOLS - SPLIT                                      # 576

_CACHE = {}


def _build_nc():
    nc = bacc.Bacc("TRN2", target_bir_lowering=False, debug=False)
    xt = nc.dram_tensor("xt", [P, B], BF16, kind="ExternalInput")   # xf^T padded
    wda = nc.dram_tensor("wda", [P, SPLIT], BF16, kind="ExternalInput")
    wdb = nc.dram_tensor("wdb", [P, NB], BF16, kind="ExternalInput")
    qh = [[nc.dram_tensor(f"qh{bt}{h}", [P, SPLIT if h == "a" else NB], BF16,
                          kind="ExternalInput") for h in "ab"]
          for bt in range(2)]
    out = nc.dram_tensor("out", [2, P], F32, kind="ExternalOutput")

    with tile.TileContext(nc) as tc:
        with (
            tc.tile_pool(name="const", bufs=1) as cpool,
            tc.tile_pool(name="ps", bufs=2, space="PSUM") as ps,
            tc.tile_pool(name="pst", bufs=1, space="PSUM") as pst,
        ):
            wda_t = cpool.tile([P, SPLIT], BF16, tag="wda")
            wdb_t = cpool.tile([P, NB], BF16, tag="wdb")
            xt_t = cpool.tile([P, B], BF16, tag="xt")
            qh_t = [[cpool.tile([P, SPLIT if h == "a" else NB], BF16,
                                tag=f"qh{bt}{h}", name=f"qh{bt}{h}_t")
                     for h in "ab"] for bt in range(2)]
            # transfers serialize across the 3 queues at the per-core HBM
            # cap, so issue in need-order: xt+wda gate the first matmul,
            # qh0a the first dot; qh1b is needed last
            nc.sync.dma_start(wda_t[:], wda[:])
            nc.sync.dma_start(wdb_t[:], wdb[:])
            nc.scalar.dma_start(xt_t[:], xt[:])
            nc.scalar.dma_start(qh_t[0][1][:], qh[0][1][:])
            nc.scalar.dma_start(qh_t[1][1][:], qh[1][1][:])
            nc.gpsimd.dma_start(qh_t[0][0][:], qh[0][0][:])
            nc.gpsimd.dma_start(qh_t[1][0][:], qh[1][0][:])

            ident = cpool.tile([P, P], F32, tag="ident")
            make_identity(nc, ident[:])

            res = cpool.tile([P, 2], F32, tag="res")
            acc = cpool.tile([P, 4], F32, tag="acc")
            e = cpool.tile([P, NB], F32, tag="e")
            for bt in range(2):
                # separate a/b PSUM tiles: per-tile dep tracking would
                # otherwise impose a false WAR between the b-matmuls and
                # the a-dot of the same batch tile
                za_ps = ps.tile([P, SPLIT], F32, tag="za", name=f"za{bt}_ps")
                zb_ps = ps.tile([P, NB], F32, tag="zb", name=f"zb{bt}_ps")
                xts = xt_t[:, bt * P:(bt + 1) * P]
                nc.tensor.matmul(za_ps[:], xts, wda_t[:],
                                 start=True, stop=True)
                nc.vector.scalar_tensor_tensor(
                    out=e[:, 0:SPLIT], in0=za_ps[:], scalar=1.0,
                    in1=qh_t[bt][0][:],
                    op0=mybir.AluOpType.mult, op1=mybir.AluOpType.mult,
                    accum_out=acc[:, 2 * bt:2 * bt + 1],
                )
                nc.tensor.matmul(zb_ps[:, 0:512], xts, wdb_t[:, 0:512],
                                 start=True, stop=True)
                nc.tensor.matmul(zb_ps[:, 512:NB], xts, wdb_t[:, 512:NB],
                                 start=True, stop=True)
                nc.vector.scalar_tensor_tensor(
                    out=e[:], in0=zb_ps[:], scalar=1.0,
                    in1=qh_t[bt][1][:],
                    op0=mybir.AluOpType.mult, op1=mybir.AluOpType.mult,
                    accum_out=acc[:, 2 * bt + 1:2 * bt + 2],
                )
            nc.vector.tensor_reduce(
                res[:], acc[:].rearrange("p (t h) -> p t h", t=2),
                axis=mybir.AxisListType.X, op=mybir.AluOpType.add,
            )
            # [128,2] -> [2,128] so the out DMA is 2 contiguous descriptors
            tps = pst.tile([2, P], F32, tag="tps")
            nc.tensor.transpose(tps[:], res[:], ident[:])
            osb = cpool.tile([2, P], F32, tag="osb")
            nc.vector.tensor_copy(osb[:], tps[:])
            # split over two queues: desc-gen runs in parallel
            nc.scalar.dma_start(out[0:1, :], osb[0:1, :])
            nc.sync.dma_start(out[1:2, :], osb[1:2, :])
    nc.compile()
    return nc


def _prep_inputs(x, weights, comb_idx):
    """Host-side layout prep: xf paddings, pair-products Q, dense weight chunks."""
    x = np.ascontiguousarray(np.asarray(x, dtype=np.float32))
    w = np.asarray(weights, dtype=np.float32).ravel()
    ci = np.asarray(comb_idx)
    i_, j_ = ci[:, 0].astype(np.int64), ci[:, 1].astype(np.int64)
    k_ = ci[:, 2].astype(np.int64)

    xf = np.concatenate([np.ones((B, 1), np.float32), x], axis=1)   # [256,127]
    xbp = np.zeros((B, P), np.float32)
    xbp[:, :NF] = xf

    xt = np.zeros((P, B), np.float32)
    xt[:NF, :] = xf.T
    xt16 = xt.astype(BF16_NP)

    # lex pair-row index of each combo
    ar = np.arange(NF, dtype=np.int64)
    rsp = ar * NF - (ar * (ar - 1)) // 2
    q = rsp[i_] + (j_ - i_)
    Wd = np.zeros((8128, NF), np.float32)
    Wd[q, k_] = w

    in_maps = []
    for c in range(8):
        big = np.zeros((P, NCOLS), np.float32)
        Q = np.zeros((B, NCOLS), np.float32)
        for t in range(NCLASS):
            i = 8 * t + c
            if i > 126:
                continue
            o = int(OFFS[t])
            Q[:, o:o + WIDTHS[t]] = xf[:, i:i + 1] * xbp[:, 8 * t:P]
            p0 = int(rsp[i])
            big[:NF, o + (i - 8 * t): o + (NF - 8 * t)] = Wd[p0:p0 + (NF - i)].T
        big16 = big.astype(BF16_NP)
        Q16 = Q.astype(BF16_NP)
        m = {
            "xt": xt16,
            "wda": np.ascontiguousarray(big16[:, 0:SPLIT]),
            "wdb": np.ascontiguousarray(big16[:, SPLIT:NCOLS]),
        }
        for bt in range(2):
            m[f"qh{bt}a"] = np.ascontiguousarray(Q16[bt * P:(bt + 1) * P, 0:SPLIT])
            m[f"qh{bt}b"] = np.ascontiguousarray(Q16[bt * P:(bt + 1) * P, SPLIT:NCOLS])
        in_maps.append(m)
    return in_maps


def _get_nc():
    if "nc" not in _CACHE:
        _CACHE["nc"] = _build_nc()
    return _CACHE["nc"]


def run_spmd(x, weights, comb_idx, trace=False):
    nc = _get_nc()
    in_maps = _prep_inputs(x, weights, comb_idx)
    res = run_bass_kernel_spmd(nc, in_maps, list(range(8)), trace=trace)
    acc = np.zeros((2, P), np.float64)
    for c in range(8):
        acc += res.results[c]["out"].astype(np.float64)
    return acc.reshape(B, 1).astype(np.float32), res


def kernel(x, weights, comb_idx):
    out, _ = run_spmd(x, weights, comb_idx, trace=False)
    return out
